# revision 1
# baseline (speedup 1.0000x reference)
"""AttentionPooling (ragged graph cross-attention pooling) on 8 TRN2 NeuronCores.

Strategy (SPMD, no collectives):
  * Host assigns 8 whole graphs to each of the 8 cores (serpentine by size),
    sorts each core's graphs by size into 8 "slots".  Slot j has a fixed tile
    count T[j] (shared by all cores, since the instruction stream is shared);
    each graph's edges are placed at its slot offset and zero-padded.
  * Host ships x^T (transposed edge features, bf16) per core + replicated
    weights.  Padding edges give exp(0)=1 in the softmax denominator, which is
    corrected with a host-computed per-slot pad count.
  * Softmax is computed without max-subtraction (scores ~ N(0,1); exp cannot
    overflow fp32) — mathematically identical to the reference's stable form.
  * Scores are linear in x: scores = (x @ w_k) . q  =  x @ Ws where
    Ws[:, (h,s)] = sum_d w_k[:, (h,d)] q[s,h,d] / sqrt(hd).  Ws ([256, 256])
    is host-precomputed from the weights and shipped fused with w_v as one
    [256, 512] operand, so the per-tile device work is:
      [v | sc][e, :] = x @ [w_v | Ws]    (PE, 2 matmuls/tile, N=512)
      ex             = exp(sc)           (ACT, psum->sbuf bf16)
      pooled[(h,s),(h,d)|denom] += ex.T @ [v | 1]  (PE, psum-accum per graph)
  * Per graph: denom -= npad; normalize by 1/denom (DVE); 32x32 block
    transpose (DVE StreamTranspose) to build the [128, (s,half)*8graphs]
    operand P2 for the MLP (w1 needs no permutation in this layout).
  * MLP: h1 = silu(pooled @ w1 + b1) (PE, 4-way tile_position-packed, +ACT),
    out = h1 @ w2 + b2 (PE), emitted as out^T [256, 8] per core; the host
    scatters core outputs into the final [64, 256].
"""

import os
import sys
from contextlib import ExitStack

import numpy as np

for _p in ("/opt/trn_rl_repo",):
    if _p not in sys.path:
        sys.path.append(_p)

import ml_dtypes  # noqa: E402

import concourse.bass as bass  # noqa: E402
import concourse.tile as tile  # noqa: E402
from concourse import mybir  # noqa: E402
from concourse.bass_utils import run_bass_kernel_spmd  # noqa: E402
from concourse.vector_clock import ScopedClock  # noqa: E402

BF16 = ml_dtypes.bfloat16

E, B, H, S, NH, HD = 131072, 64, 256, 32, 8, 32
NCORES = 8
NG = B // NCORES        # graphs (slots) per core
TILE = 128              # edge tile
GROUP = 512             # x^T DMA chunk (4 tiles)
SCALE = 1.0 / float(np.sqrt(HD))

AF = mybir.ActivationFunctionType

# ---------------------------------------------------------------------------
# Walrus workaround: this toolchain's InstDrain accepts only ONE sync wait;
# Tile's kernel-tail drain carries one wait per outstanding semaphore.
# Split it into a chain of single-wait drains.
_MAXW = 1


def _split_drain_and_barrier(self, tick_clock, wait_clock):
    nc = self.nc
    drain_inst = nc.sync.drain()
    wait_clock.add_sem_waits(
        drain_inst.ins, ScopedClock({None: tick_clock.global_clock})
    )
    waits = list(drain_inst.ins.sync_info.on_wait)
    if len(waits) > _MAXW:
        drain_inst.ins.sync_info = mybir.SyncInfo(on_wait=waits[:_MAXW], on_update=[])
        for i in range(_MAXW, len(waits), _MAXW):
            d2 = nc.sync.drain()
            d2.ins.sync_info = mybir.SyncInfo(
                on_wait=waits[i : i + _MAXW], on_update=[]
            )
    nc.all_engine_barrier()
    popped = nc._tile_sem_poison_stack.pop()
    assert popped is self._sem_poison
    nc.clear_and_free_semaphores(list(self.sems.allocated().values()))
    nc.all_engine_barrier()


tile.TileContext._drain_and_barrier = _split_drain_and_barrier

# Engine instructions are capped at 2 sync waits by this walrus (Drain/NoOp
# at 1).  Tile's sem-assignment occasionally emits more.  Hoist the excess
# onto single-wait NoOps inserted just before, on the same engine — the
# engine stalls at the NoOp instead, which is semantically identical.
_WAIT_CAP = {"InstDrain": 1}
_WAIT_CAP_DEFAULT = 1


def _fix_excess_waits(nc):
    n_fixed = 0
    for fn in nc.m.functions:
        for bb in fn.blocks:
            insts = bb.instructions
            out = []
            changed = False
            for inst in insts:
                si = inst.sync_info
                waits = list(si.on_wait) if si is not None else []
                cap = _WAIT_CAP.get(type(inst).__name__, _WAIT_CAP_DEFAULT)
                if len(waits) > cap:
                    changed = True
                    n_fixed += 1
                    excess = waits[: len(waits) - cap]
                    for i, w in enumerate(excess):
                        nop = mybir.InstNoOp(
                            name=f"{inst.name}-hw{i}", ins=[], outs=[]
                        )
                        nop.engine = inst.engine
                        nop.sync_info = mybir.SyncInfo(on_wait=[w], on_update=[])
                        out.append(nop)
                    inst.sync_info = mybir.SyncInfo(
                        on_wait=waits[len(excess) :], on_update=list(si.on_update)
                    )
                out.append(inst)
            if changed:
                bb.instructions = out
    return n_fixed

# ---------------------------------------------------------------------------

_PROGRAM_CACHE: dict[tuple, "bass.Bass"] = {}
LAST_RESULTS = None  # BassKernelResults of the most recent run (for testing)


def _install_ntff_hook_shim():
    """The image's antenv lacks axon_hooks; recreate it so trace=True works."""
    try:
        import types

        import antenv

        if "antenv.axon_hooks" not in sys.modules:
            mod = types.ModuleType("antenv.axon_hooks")
            mod._hook = None

            def set_axon_ntff_profile_hook(h):
                mod._hook = h

            def get_axon_ntff_profile_hook():
                return mod._hook

            mod.set_axon_ntff_profile_hook = set_axon_ntff_profile_hook
            mod.get_axon_ntff_profile_hook = get_axon_ntff_profile_hook
            sys.modules["antenv.axon_hooks"] = mod
            antenv.axon_hooks = mod
        import antenv.axon_hooks as ah

        if ah.get_axon_ntff_profile_hook() is None:
            from trn_agent_boot.trn_boot import _ntff_profile_via_ctypes

            ah.set_axon_ntff_profile_hook(
                _ntff_profile_via_ctypes("/opt/axon/libaxon_pjrt.so")
            )
    except Exception:
        pass


_install_ntff_hook_shim()

# Optional experiment: let walrus double-buffer LDWEIGHTS (default off here).
import concourse.bass_utils as _bass_utils  # noqa: E402

_orig_run_command = _bass_utils.run_command


def _run_command_ldwopt(cmd, **kw):
    if isinstance(cmd, list):
        cmd = [
            "--enable-ldw-opt=true" if c == "--enable-ldw-opt=false" else c
            for c in cmd
        ]
    return _orig_run_command(cmd, **kw)


if os.environ.get("KERNEL_LDW_OPT") == "1":
    _bass_utils.run_command = _run_command_ldwopt


def build_program(slot_tiles: tuple[int, ...]) -> "bass.Bass":
    """Build the SPMD Bass program for per-core slot tile counts."""
    TT = sum(slot_tiles)
    EC = TT * TILE
    assert TT % (GROUP // TILE) == 0
    NGRP = TT // (GROUP // TILE)

    # per-tile slot id / first / last flags
    slot_of, first_of, last_of = [], [], []
    for j, tj in enumerate(slot_tiles):
        for t in range(tj):
            slot_of.append(j)
            first_of.append(t == 0)
            last_of.append(t == tj - 1)

    f32, bf16 = mybir.dt.float32, mybir.dt.bfloat16
    nc = bass.Bass("TRN2", target_bir_lowering=False, debug=False, num_devices=NCORES)

    xt_d = nc.dram_tensor("xt", [H, EC], bf16, kind="ExternalInput").ap()
    wvs_d = nc.dram_tensor("wvs", [H, 2 * H], bf16, kind="ExternalInput").ap()
    w1_d = nc.dram_tensor("w1", [S * H, H], bf16, kind="ExternalInput").ap()
    w2_d = nc.dram_tensor("w2", [H, H], bf16, kind="ExternalInput").ap()
    b1_d = nc.dram_tensor("b1", [NG, H], f32, kind="ExternalInput").ap()
    b2_d = nc.dram_tensor("b2", [H, 1], f32, kind="ExternalInput").ap()
    npad_d = nc.dram_tensor("npad", [128, NG], f32, kind="ExternalInput").ap()
    ident_d = nc.dram_tensor("ident", [128, 128], bf16, kind="ExternalInput").ap()
    qsel_d = nc.dram_tensor("qsel", [128, NG], bf16, kind="ExternalInput").ap()
    outT_d = nc.dram_tensor("outT", [H, NG], f32, kind="ExternalOutput").ap()

    with tile.TileContext(nc) as tc, ExitStack() as ctx:
        const = ctx.enter_context(tc.tile_pool(name="const", bufs=1))
        w2_sb = const.tile([128, 2 * H], bf16)
        wvs_sb = const.tile([128, 2 * 2 * H], bf16)  # k-tile k: [wv_k | ws_k]
        w1_sb = const.tile([128, 64 * H], bf16)
        ident_sb = const.tile([128, 128], bf16)
        qsel_sb = const.tile([128, NG], bf16)
        b1_sb = const.tile([NG, H], f32)
        b2_sb = const.tile([128, 2], f32)
        npad_sb = const.tile([128, NG], f32)
        P2 = const.tile([128, 64 * NG], bf16)

        for k in range(2):
            r = slice(k * 128, (k + 1) * 128)
            nc.scalar.dma_start(wvs_sb[:, k * 2 * H : (k + 1) * 2 * H], wvs_d[r, :])
        nc.scalar.dma_start(npad_sb[:], npad_d[:])
        for k in range(2):
            r = slice(k * 128, (k + 1) * 128)
            nc.scalar.dma_start(w2_sb[:, k * H : (k + 1) * H], w2_d[r, :])
            nc.scalar.dma_start(b2_sb[:, k : k + 1], b2_d[r, :])
        nc.scalar.dma_start(ident_sb[:], ident_d[:])
        nc.scalar.dma_start(qsel_sb[:], qsel_d[:])
        nc.scalar.dma_start(b1_sb[:], b1_d[:])

        # Warm the ACT function tables while the first DMAs are in flight,
        # so the table loads are off the critical path.
        warm = const.tile([1, 2], f32)
        nc.gpsimd.memset(warm[:, 0:1], 0.0)
        nc.scalar.activation(warm[:, 1:2], warm[:, 0:1], AF.Exp)
        nc.scalar.activation(warm[:, 1:2], warm[:, 0:1], AF.Sigmoid)

        # ---- main edge loop ---------------------------------------------
        xt_pool = ctx.enter_context(tc.tile_pool(name="xtp", bufs=4))
        ex_pool = ctx.enter_context(tc.tile_pool(name="exp", bufs=6))
        ext_pool = ctx.enter_context(tc.tile_pool(name="ext", bufs=2))

        NRING = 6
        vs_ring = [const.tile([128, 258], bf16, name=f"vsring{i}") for i in range(NRING)]
        for t in vs_ring:
            nc.vector.memset(t[:, 128:129], 1.0)
            nc.vector.memset(t[:, 257:258], 1.0)

        pooled_tiles: list = [None, None]

        def emit_pooled(sl, fi, la, ex, vs):
            if fi:
                pooled_tiles[0] = pl_pool.tile([128, 129], f32, tag="pl0", name=f"pl0_s{sl}")
                pooled_tiles[1] = pl_pool.tile([128, 129], f32, tag="pl1", name=f"pl1_s{sl}")
            for m in range(2):
                nc.tensor.matmul(
                    pooled_tiles[m][:],
                    ex[:, m * 128 : (m + 1) * 128],
                    vs[:, m * 129 : m * 129 + 129],
                    start=fi,
                    stop=la,
                )
            if la:
                extract_graph(sl, pooled_tiles)

        P2v = P2[:].rearrange("p (s x) -> p s x", x=2 * NG)

        def extract_graph(g, ptiles):
            copy_eng = nc.vector if g == NG - 1 else nc.gpsimd
            for m in range(2):
                den = ext_pool.tile([128, 1], f32, tag="den", name=f"den{g}_{m}")
                nc.vector.tensor_scalar_sub(
                    den[:], ptiles[m][:, 128:129], npad_sb[:, g : g + 1]
                )
                rec = ext_pool.tile([128, 1], f32, tag="rec", name=f"rec{g}_{m}")
                nc.vector.reciprocal(rec[:], den[:])
                pn = ext_pool.tile([128, 128], f32, tag="pn", name=f"pn{g}_{m}")
                nc.vector.tensor_scalar_mul(pn[:], ptiles[m][:, 0:128], rec[:])
                pt = ext_pool.tile([128, 128], f32, tag="pt", name=f"pt{g}_{m}")
                nc.vector.transpose(pt[:], pn[:])
                for hh in range(4):
                    rr = slice(hh * 32, (hh + 1) * 32)
                    src = pt[rr, hh * 32 : (hh + 1) * 32].rearrange(
                        "p (a o) -> p a o", o=1
                    )
                    copy_eng.tensor_copy(P2v[rr, :, m * NG + g : m * NG + g + 1], src)

        with (
            tc.tile_pool(name="vscp", bufs=3, space="PSUM") as vsc_pool,
            tc.tile_pool(name="plp", bufs=2, space="PSUM") as pl_pool,
        ):
            from collections import deque

            pending = deque()
            tidx = 0
            for grp in range(NGRP):
                xt = [
                    xt_pool.tile([128, GROUP], bf16, tag="xt", name=f"xt_{grp}_{i}")
                    for i in range(2)
                ]
                for k in range(2):
                    nc.sync.dma_start(
                        xt[k][:],
                        xt_d[k * 128 : (k + 1) * 128, grp * GROUP : (grp + 1) * GROUP],
                    )
                for sub in range(4):
                    sl, fi, la = slot_of[tidx], first_of[tidx], last_of[tidx]
                    e0 = sub * TILE
                    vsc = vsc_pool.tile([128, 512], f32, tag="vsc", name=f"vsc{tidx}")
                    for k in range(2):
                        nc.tensor.matmul(
                            vsc[:],
                            xt[k][:, e0 : e0 + TILE],
                            wvs_sb[:, k * 2 * H : (k + 1) * 2 * H],
                            start=(k == 0),
                            stop=(k == 1),
                        )
                    ex = ex_pool.tile([128, 256], bf16, tag="ex", name=f"ex{tidx}")
                    nc.scalar.activation(ex[:], vsc[:, H : 2 * H], AF.Exp)
                    vs = vs_ring[tidx % NRING]
                    nc.vector.tensor_copy(
                        vs[:].rearrange("p (b c) -> p b c", c=129)[:, :, 0:128],
                        vsc[:, 0:H].rearrange("p (b c) -> p b c", c=128),
                    )
                    pending.append((sl, fi, la, ex, vs))
                    while len(pending) > 2:
                        emit_pooled(*pending.popleft())
                    tidx += 1
            while pending:
                emit_pooled(*pending.popleft())

        # w1 load — one big blocked DMA on the Scalar HWDGE ring, so its 4MB
        # transfer cannot queue ahead of the edge-loop xt groups on the Sync
        # ring (the scheduler hoists it regardless of trace position).
        nc.scalar.dma_start(
            w1_sb[:].rearrange("p (k c) -> p k c", c=H),
            w1_d[:].rearrange("(k p) c -> p k c", p=128),
        )

        # ---- MLP tail ----------------------------------------------------
        with (
            tc.tile_pool(name="mlpp", bufs=2, space="PSUM") as mp,
            tc.tile_pool(name="mlps", bufs=2) as ms,
        ):
            h1pp = mp.tile([128, H], f32, tag="h1pp")
            for j in range(64):
                q = j % 4
                nc.tensor.matmul(
                    h1pp[q * 32 : q * 32 + NG, :],
                    P2[:, j * NG : (j + 1) * NG],
                    w1_sb[:, j * H : (j + 1) * H],
                    start=(j < 4),
                    stop=(j >= 60),
                    tile_position=(0, q * 32),
                    skip_group_check=True,
                )
            h1ps = ms.tile([128, H], bf16, tag="h1ps")
            nc.gpsimd.memset(h1ps[:], 0.0)
            for q in range(4):
                eng = nc.vector if q % 2 == 0 else nc.scalar
                if eng is nc.vector:
                    nc.vector.tensor_copy(
                        h1ps[q * 32 : q * 32 + NG, :], h1pp[q * 32 : q * 32 + NG, :]
                    )
                else:
                    nc.scalar.activation(
                        h1ps[q * 32 : q * 32 + NG, :],
                        h1pp[q * 32 : q * 32 + NG, :],
                        AF.Copy,
                    )
            h1p = mp.tile([NG, H], f32, tag="h1p")
            nc.tensor.matmul(h1p[:], qsel_sb[:], h1ps[:], start=True, stop=True)
            h1s = ms.tile([NG, H], f32, tag="h1s")
            nc.vector.tensor_add(h1s[:], h1p[:], b1_sb[:])
            h1g = ms.tile([NG, H], f32, tag="h1g")
            nc.scalar.activation(h1g[:], h1s[:], AF.Sigmoid)
            h1b = ms.tile([NG, H], bf16, tag="h1b")
            nc.vector.tensor_mul(h1b[:], h1s[:], h1g[:])
            h1t = []
            for m in range(2):
                h1tp = mp.tile([128, NG], bf16, tag="h1tp", name=f"h1tp{m}")
                nc.tensor.transpose(
                    h1tp[:], h1b[:, m * 128 : (m + 1) * 128], ident_sb[0:NG, 0:NG]
                )
                ht = ms.tile([128, NG], bf16, tag=f"h1t{m}")
                nc.vector.tensor_copy(ht[:], h1tp[:])
                h1t.append(ht)
            for m in range(2):
                otp = mp.tile([128, NG], f32, tag="otp", name=f"otp{m}")
                for k in range(2):
                    nc.tensor.matmul(
                        otp[:],
                        w2_sb[:, k * H + m * 128 : k * H + m * 128 + 128],
                        h1t[k][:],
                        start=(k == 0),
                        stop=(k == 1),
                    )
                osb = ms.tile([128, NG], f32, tag="osb", name=f"osb{m}")
                nc.vector.tensor_scalar_add(osb[:], otp[:], b2_sb[:, m : m + 1])
                nc.sync.dma_start(outT_d[m * 128 : (m + 1) * 128, :], osb[:])

    return nc


def get_program(slot_tiles: tuple[int, ...]) -> "bass.Bass":
    if slot_tiles not in _PROGRAM_CACHE:
        nc = build_program(slot_tiles)
        # HW-path only (CoreSim snapshots the program before this pass)
        _fix_excess_waits(nc)
        _PROGRAM_CACHE[slot_tiles] = nc
    return _PROGRAM_CACHE[slot_tiles]


# ---------------------------------------------------------------------------
# Host-side sharding / padding


def plan_shards(batch: np.ndarray):
    """Returns (assign [NCORES][NG] graph ids, slot_tiles tuple, sizes)."""
    sizes = np.bincount(batch, minlength=B).astype(np.int64)
    order = np.argsort(-sizes, kind="stable")
    assign = [[] for _ in range(NCORES)]
    for r in range(NG):
        row = order[r * NCORES : (r + 1) * NCORES]
        if r % 2 == 1:
            row = row[::-1]
        for c in range(NCORES):
            assign[c].append(int(row[c]))
    for c in range(NCORES):
        assign[c].sort(key=lambda g: -sizes[g])
    slot_tiles = []
    for j in range(NG):
        mx = max(sizes[assign[c][j]] for c in range(NCORES))
        slot_tiles.append(int(max(1, -(-mx // TILE))))
    # round total tiles up to a GROUP multiple (pad goes to the last slot)
    rem = (-sum(slot_tiles)) % (GROUP // TILE)
    slot_tiles[-1] += rem
    return assign, tuple(slot_tiles), sizes


def make_in_maps(edge_features, batch, seed_vectors, w_q, w_k, w_v, w1, b1, w2, b2):
    edge_features = np.asarray(edge_features, dtype=np.float32)
    batch = np.asarray(batch)
    assign, slot_tiles, sizes = plan_shards(batch)
    TT = sum(slot_tiles)
    EC = TT * TILE

    starts = np.searchsorted(batch, np.arange(B))
    xb = edge_features.astype(BF16)

    # Ws[hin, h*S+s] = sum_d w_k[hin, h*HD+d] * q[s, h, d] / sqrt(HD)
    q = (np.asarray(seed_vectors, np.float32) @ np.asarray(w_q, np.float32)).reshape(
        S, NH, HD
    )
    wk3 = np.asarray(w_k, np.float32).reshape(H, NH, HD)
    Ws = (np.einsum("ihd,shd->ihs", wk3, q) * SCALE).reshape(H, NH * S)
    wvs = np.concatenate([np.asarray(w_v, np.float32), Ws], axis=1)

    shared = {
        "wvs": np.ascontiguousarray(wvs.astype(BF16)),
        "w1": np.ascontiguousarray(np.asarray(w1).astype(BF16)),
        "w2": np.ascontiguousarray(np.asarray(w2).astype(BF16)),
        "b1": np.ascontiguousarray(
            np.broadcast_to(np.asarray(b1, dtype=np.float32), (NG, H))
        ),
        "b2": np.ascontiguousarray(np.asarray(b2, dtype=np.float32).reshape(H, 1)),
        "ident": np.eye(128, dtype=BF16),
        "qsel": np.ascontiguousarray(
            (np.arange(128)[:, None] % 32 == np.arange(NG)[None, :]).astype(BF16)
        ),
    }

    in_maps = []
    for c in range(NCORES):
        xt = np.zeros((H, EC), dtype=BF16)
        npad = np.zeros(NG, dtype=np.float32)
        off = 0
        for j, g in enumerate(assign[c]):
            n = int(sizes[g])
            xt[:, off : off + n] = xb[starts[g] : starts[g] + n].T
            npad[j] = slot_tiles[j] * TILE - n
            off += slot_tiles[j] * TILE
        m = dict(shared)
        m["xt"] = xt
        m["npad"] = np.ascontiguousarray(np.broadcast_to(npad, (128, NG)))
        in_maps.append(m)
    return in_maps, assign, slot_tiles


def kernel(
    edge_features,
    edge_coords,
    batch,
    seed_vectors,
    w_q,
    w_k,
    w_v,
    w1,
    b1,
    w2,
    b2,
):
    in_maps, assign, slot_tiles = make_in_maps(
        edge_features, batch, seed_vectors, w_q, w_k, w_v, w1, b1, w2, b2
    )
    nc = get_program(slot_tiles)

    res = run_bass_kernel_spmd(nc, in_maps, core_ids=list(range(NCORES)))
    global LAST_RESULTS
    LAST_RESULTS = res

    out = np.zeros((B, H), dtype=np.float32)
    for c in range(NCORES):
        outT = res.results[c]["outT"]  # [H, NG]
        for j, g in enumerate(assign[c]):
            out[g, :] = outT[:, j]
    return out



# revision 7
# speedup vs baseline: 1.4075x; 1.4075x over previous
"""AttentionPooling (ragged graph cross-attention pooling) on 8 TRN2 NeuronCores.

v2 strategy (SPMD, no collectives) — "x-pooling" restructure:
  * Host assigns 8 whole graphs per core (serpentine by size), sorted into 8
    slots; per-slot tile counts are shared across cores (shared instruction
    stream); edges zero-padded to the slot size.
  * Linearity trick: pooled_v = (sum_e w[e,s,h] * x_e) @ w_v — pool the RAW
    edge features with the attention weights, apply w_v once per slot on the
    tiny pooled matrix.  This removes the per-edge V projection (PE) and the
    per-tile PSUM->SBUF V copy (DVE) entirely.
  * Per 128-edge tile the device does:
      scores = x^T-tile @ Ws           (PE, 2 matmuls N=256, psum [e,256])
      ex     = exp(scores)             (ACT, one [128,512] EXP per 2 tiles)
      xpool += ex^T @ [x | 1]          (PE, 2 matmuls N=257, psum-accum/slot)
    where Ws = w_k @ q^T / sqrt(hd) is host-folded so scores need no separate
    K projection, and the baked-in 1s column yields the softmax denominator
    (corrected for padding with a host-computed per-slot pad count).
  * Per slot: normalize by 1/(denom-npad) (DVE, cast bf16), 4 PE transposes,
    apply w_v quadrants (4 matmuls N=128) -> pov^T [hd, sh], scatter the
    block-diagonal into the MLP operand P2.
  * MLP: h1 = silu(pooled @ w1 + b1) (PE 64 j-blocks, 4-way tile_position),
    out = h1 @ w2 + b2 emitted row-contiguous [NG, H]; host scatters.
  * All bulk DMA is host-pre-tiled so every transfer is >=2KB contiguous per
    partition and needs ONE trigger per group per stream.
  * A chain of dummy matmuls at kernel start ramps the PE p-state to full
    clock before the first real tile arrives.
"""

import os
import sys
from collections import deque
from contextlib import ExitStack

import numpy as np

for _p in ("/opt/trn_rl_repo",):
    if _p not in sys.path:
        sys.path.append(_p)

import ml_dtypes  # noqa: E402

import concourse.bass as bass  # noqa: E402
import concourse.tile as tile  # noqa: E402
from concourse import mybir  # noqa: E402
from concourse.bass_utils import run_bass_kernel_spmd  # noqa: E402
from concourse.vector_clock import ScopedClock  # noqa: E402

BF16 = ml_dtypes.bfloat16

E, B, H, S, NH, HD = 131072, 64, 256, 32, 8, 32
NCORES = 8
NG = B // NCORES        # graphs (slots) per core
TILE = 128              # edge tile
GROUP = 512             # edges per DMA group (4 tiles)
SCALE = 1.0 / float(np.sqrt(HD))
WARM_MM = 12            # PE p-state warmup matmuls

AF = mybir.ActivationFunctionType

# ---------------------------------------------------------------------------
# Walrus workaround: this toolchain's InstDrain accepts only ONE sync wait;
# Tile's kernel-tail drain carries one wait per outstanding semaphore.
# Split it into a chain of single-wait drains.
_MAXW = 1


def _split_drain_and_barrier(self, tick_clock, wait_clock):
    nc = self.nc
    drain_inst = nc.sync.drain()
    wait_clock.add_sem_waits(
        drain_inst.ins, ScopedClock({None: tick_clock.global_clock})
    )
    waits = list(drain_inst.ins.sync_info.on_wait)
    if len(waits) > _MAXW:
        drain_inst.ins.sync_info = mybir.SyncInfo(on_wait=waits[:_MAXW], on_update=[])
        for i in range(_MAXW, len(waits), _MAXW):
            d2 = nc.sync.drain()
            d2.ins.sync_info = mybir.SyncInfo(
                on_wait=waits[i : i + _MAXW], on_update=[]
            )
    nc.all_engine_barrier()
    popped = nc._tile_sem_poison_stack.pop()
    assert popped is self._sem_poison
    nc.clear_and_free_semaphores(list(self.sems.allocated().values()))
    nc.all_engine_barrier()


tile.TileContext._drain_and_barrier = _split_drain_and_barrier

# Engine instructions are capped at 2 sync waits by this walrus (Drain/NoOp
# at 1).  Tile's sem-assignment occasionally emits more.  Hoist the excess
# onto single-wait NoOps inserted just before, on the same engine.
_WAIT_CAP = {"InstDrain": 1}
_WAIT_CAP_DEFAULT = 1


def _fix_excess_waits(nc):
    n_fixed = 0
    for fn in nc.m.functions:
        for bb in fn.blocks:
            insts = bb.instructions
            out = []
            changed = False
            for inst in insts:
                si = inst.sync_info
                waits = list(si.on_wait) if si is not None else []
                cap = _WAIT_CAP.get(type(inst).__name__, _WAIT_CAP_DEFAULT)
                if len(waits) > cap:
                    changed = True
                    n_fixed += 1
                    excess = waits[: len(waits) - cap]
                    for i, w in enumerate(excess):
                        nop = mybir.InstNoOp(
                            name=f"{inst.name}-hw{i}", ins=[], outs=[]
                        )
                        nop.engine = inst.engine
                        nop.sync_info = mybir.SyncInfo(on_wait=[w], on_update=[])
                        out.append(nop)
                    inst.sync_info = mybir.SyncInfo(
                        on_wait=waits[len(excess) :], on_update=list(si.on_update)
                    )
                out.append(inst)
            if changed:
                bb.instructions = out
    return n_fixed


# ---------------------------------------------------------------------------

_PROGRAM_CACHE: dict[tuple, "bass.Bass"] = {}
LAST_RESULTS = None  # BassKernelResults of the most recent run (for testing)


def _install_ntff_hook_shim():
    """The image's antenv lacks axon_hooks; recreate it so trace=True works."""
    try:
        import types

        import antenv

        if "antenv.axon_hooks" not in sys.modules:
            mod = types.ModuleType("antenv.axon_hooks")
            mod._hook = None

            def set_axon_ntff_profile_hook(h):
                mod._hook = h

            def get_axon_ntff_profile_hook():
                return mod._hook

            mod.set_axon_ntff_profile_hook = set_axon_ntff_profile_hook
            mod.get_axon_ntff_profile_hook = get_axon_ntff_profile_hook
            sys.modules["antenv.axon_hooks"] = mod
            antenv.axon_hooks = mod
        import antenv.axon_hooks as ah

        if ah.get_axon_ntff_profile_hook() is None:
            from trn_agent_boot.trn_boot import _ntff_profile_via_ctypes

            ah.set_axon_ntff_profile_hook(
                _ntff_profile_via_ctypes("/opt/axon/libaxon_pjrt.so")
            )
    except Exception:
        pass


_install_ntff_hook_shim()


def build_program(slot_tiles: tuple[int, ...]) -> "bass.Bass":
    """Build the SPMD Bass program for per-core slot tile counts."""
    TT = sum(slot_tiles)
    assert TT % 4 == 0
    NGRP = TT // 4

    # per-tile slot id / first / last flags
    slot_of, first_of, last_of = [], [], []
    for j, tj in enumerate(slot_tiles):
        for t in range(tj):
            slot_of.append(j)
            first_of.append(t == 0)
            last_of.append(t == tj - 1)

    f32, bf16 = mybir.dt.float32, mybir.dt.bfloat16
    nc = bass.Bass("TRN2", target_bir_lowering=False, debug=False, num_devices=NCORES)

    # host-pre-tiled inputs (all >=2KB contiguous per partition per group)
    xt_d = nc.dram_tensor("xt", [128, NGRP * 1024], bf16, kind="ExternalInput").ap()
    xr_d = nc.dram_tensor("xr", [128, NGRP * 4 * (H + 1)], bf16, kind="ExternalInput").ap()
    ws_d = nc.dram_tensor("ws", [128, 2 * H], bf16, kind="ExternalInput").ap()
    wvq_d = nc.dram_tensor("wvq", [128, 4 * 128], bf16, kind="ExternalInput").ap()
    w1_d = nc.dram_tensor("w1", [128, 64 * H], bf16, kind="ExternalInput").ap()
    w2_d = nc.dram_tensor("w2", [128, 2 * H], bf16, kind="ExternalInput").ap()
    b1_d = nc.dram_tensor("b1", [NG, H], f32, kind="ExternalInput").ap()
    b2_d = nc.dram_tensor("b2", [NG, H], f32, kind="ExternalInput").ap()
    npad_d = nc.dram_tensor("npad", [128, NG], f32, kind="ExternalInput").ap()
    ident_d = nc.dram_tensor("ident", [128, 128], bf16, kind="ExternalInput").ap()
    qsel_d = nc.dram_tensor("qsel", [128, NG], bf16, kind="ExternalInput").ap()
    out_d = nc.dram_tensor("out", [NG, H], f32, kind="ExternalOutput").ap()

    XRW = H + 1  # 257: x tile width incl. baked-in 1s column

    with tile.TileContext(nc) as tc, ExitStack() as ctx:
        const = ctx.enter_context(tc.tile_pool(name="const", bufs=1))
        ws_sb = const.tile([128, 2 * H], bf16)
        wvq_sb = const.tile([128, 4 * 128], bf16)
        w1_sb = const.tile([128, 64 * H], bf16)
        w2_sb = const.tile([128, 2 * H], bf16)
        ident_sb = const.tile([128, 128], bf16)
        qsel_sb = const.tile([128, NG], bf16)
        b1_sb = const.tile([NG, H], f32)
        b2_sb = const.tile([NG, H], f32)
        npad_sb = const.tile([128, NG], f32)
        P2 = const.tile([128, 32 * 2 * NG], bf16)

        # PE p-state warmup: a chain of dummy matmuls keeps the PE busy (and
        # ramping to full clock) while the first input DMAs are in flight.
        wz = const.tile([128, 512], bf16)
        nc.gpsimd.memset(wz[:], 0.0)

        # first-needed consts on the scalar DGE ring
        nc.scalar.dma_start(ws_sb[:], ws_d[:])
        nc.scalar.dma_start(npad_sb[:], npad_d[:])

        # ACT table warm (exp + sigmoid) while DMAs fly
        warm = const.tile([1, 2], f32)
        nc.gpsimd.memset(warm[:, 0:1], 0.0)
        nc.scalar.activation(warm[:, 1:2], warm[:, 0:1], AF.Exp)
        nc.scalar.activation(warm[:, 1:2], warm[:, 0:1], AF.Sigmoid)

        nc.scalar.dma_start(wvq_sb[:], wvq_d[:])
        nc.scalar.dma_start(ident_sb[:], ident_d[:])
        nc.scalar.dma_start(qsel_sb[:], qsel_d[:])
        nc.scalar.dma_start(b1_sb[:], b1_d[:])
        nc.scalar.dma_start(b2_sb[:], b2_d[:])
        nc.scalar.dma_start(w2_sb[:], w2_d[:])

        with tc.tile_pool(name="warmp", bufs=1, space="PSUM") as wp_pool:
            wp = wp_pool.tile([128, 512], f32)
            for i in range(WARM_MM):
                nc.tensor.matmul(wp[:], wz[:, 0:128], wz[:], start=True, stop=True)

        # input rings (manual, so buffers persist and deps are per-buffer)
        NRG = 3  # groups in flight
        xtg_ring = [const.tile([128, 2, 512], bf16, name=f"xtg{i}") for i in range(NRG)]
        xrg_ring = [
            const.tile([128, 4, XRW], bf16, name=f"xrg{i}") for i in range(NRG)
        ]

        ex_pool = ctx.enter_context(tc.tile_pool(name="exp", bufs=3))
        ext_pool = ctx.enter_context(tc.tile_pool(name="ext", bufs=2))

        xp_tiles: list = [None, None]

        with (
            tc.tile_pool(name="scp", bufs=2, space="PSUM") as sc_pool,
            tc.tile_pool(name="xpp", bufs=2, space="PSUM") as xp_pool,
            tc.tile_pool(name="tpp", bufs=1, space="PSUM") as tp_pool,
            tc.tile_pool(name="pvp", bufs=1, space="PSUM") as pv_pool,
        ):
            P2v = P2[:].rearrange("p (s x) -> p s x", x=2 * NG)

            def extract_stage1(g, xp):
                """Per-slot DVE work right after the slot's last pooling MM:
                denominator, reciprocal, normalize+cast."""
                pns = []
                for m in range(2):
                    den = ext_pool.tile([128, 1], f32, tag="den", name=f"den{g}_{m}")
                    nc.vector.tensor_scalar_sub(
                        den[:], xp[m][:, H : H + 1], npad_sb[:, g : g + 1]
                    )
                    rec = ext_pool.tile([128, 1], f32, tag="rec", name=f"rec{g}_{m}")
                    nc.vector.reciprocal(rec[:], den[:])
                    pn = ext_pool.tile([128, 256], bf16, tag=f"pn{m}", name=f"pn{g}_{m}")
                    nc.vector.tensor_scalar_mul(pn[:], xp[m][:, 0:256], rec[:])
                    pns.append(pn)
                return pns

            def extract_stage2(g, pns):
                """Deferred PE work (so the FIFO PE queue never waits on the
                DVE normalize): transpose, apply w_v quadrants, scatter P2."""
                tps = tp_pool.tile([128, 512], bf16, tag="tps", name=f"tps{g}")
                for m in range(2):
                    for k in range(2):
                        q = m * 2 + k
                        nc.tensor.transpose(
                            tps[:, q * 128 : (q + 1) * 128],
                            pns[m][:, k * 128 : (k + 1) * 128],
                            ident_sb[:],
                        )
                xpT = []
                for m in range(2):
                    row = []
                    for k in range(2):
                        q = m * 2 + k
                        t_sb = ext_pool.tile(
                            [128, 128], bf16, tag=f"xpT{q}", name=f"xpT{g}_{q}"
                        )
                        nc.vector.tensor_copy(t_sb[:], tps[:, q * 128 : (q + 1) * 128])
                        row.append(t_sb)
                    xpT.append(row)
                return g, xpT

            def extract_stage3(g, xpT):
                pov = pv_pool.tile([128, 256], f32, tag="pov", name=f"pov{g}")
                for m in range(2):
                    for k in range(2):
                        nc.tensor.matmul(
                            pov[:, m * 128 : (m + 1) * 128],
                            wvq_sb[:, (k * 2 + m) * 128 : (k * 2 + m + 1) * 128],
                            xpT[m][k][:],
                            start=(k == 0),
                            stop=(k == 1),
                        )
                pv_sb = ext_pool.tile([128, 256], bf16, tag="pv", name=f"pv{g}")
                nc.vector.tensor_copy(pv_sb[:], pov[:])
                copy_eng = nc.vector if g == NG - 1 else nc.gpsimd
                for m in range(2):
                    for hh in range(4):
                        rr = slice(hh * 32, (hh + 1) * 32)
                        src = pv_sb[
                            rr, m * 128 + hh * 32 : m * 128 + (hh + 1) * 32
                        ].rearrange("p (a o) -> p a o", o=1)
                        copy_eng.tensor_copy(P2v[rr, :, m * NG + g : m * NG + g + 1], src)

            npooled = 0
            ext_queue = deque()  # (emitted-at-count, stage, payload)

            def pump_extracts(limit):
                while ext_queue and npooled - ext_queue[0][0] >= limit:
                    at, stage, payload = ext_queue.popleft()
                    if stage == 1:
                        g, pns = payload
                        ext_queue.append((npooled, 2, extract_stage2(g, pns)))
                    else:
                        extract_stage3(*payload)

            def emit_pooled(sl, fi, la, ex_t, half, xr_t, sub):
                nonlocal npooled
                if fi:
                    xp_tiles[0] = xp_pool.tile(
                        [128, H + 1], f32, tag="xp0", name=f"xp0_s{sl}"
                    )
                    xp_tiles[1] = xp_pool.tile(
                        [128, H + 1], f32, tag="xp1", name=f"xp1_s{sl}"
                    )
                for m in range(2):
                    nc.tensor.matmul(
                        xp_tiles[m][:],
                        ex_t[:, half * 256 + m * 128 : half * 256 + (m + 1) * 128],
                        xr_t[:, sub, :],
                        start=fi,
                        stop=la,
                    )
                npooled += 1
                if la:
                    pns = extract_stage1(sl, xp_tiles)
                    ext_queue.append((npooled, 1, (sl, pns)))
                pump_extracts(3)

            pending = deque()
            sc_pair = None
            tidx = 0
            for grp in range(NGRP):
                xtg = xtg_ring[grp % NRG]
                xrg = xrg_ring[grp % NRG]
                nc.sync.dma_start(
                    xtg[:],
                    xt_d[:, grp * 1024 : (grp + 1) * 1024].rearrange(
                        "p (k c) -> p k c", k=2
                    ),
                )
                nc.sync.dma_start(
                    xrg[:],
                    xr_d[:, grp * 4 * XRW : (grp + 1) * 4 * XRW].rearrange(
                        "p (t c) -> p t c", t=4
                    ),
                )
                for sub in range(4):
                    half = tidx % 2
                    if half == 0:
                        sc_pair = sc_pool.tile(
                            [128, 512], f32, tag="sc", name=f"sc{tidx}"
                        )
                    for k in range(2):
                        nc.tensor.matmul(
                            sc_pair[:, half * 256 : (half + 1) * 256],
                            xtg[:, k, sub * TILE : (sub + 1) * TILE],
                            ws_sb[:, k * 256 : (k + 1) * 256],
                            start=(k == 0),
                            stop=(k == 1),
                        )
                    if half == 1:
                        ex_t = ex_pool.tile([128, 512], bf16, tag="ex", name=f"ex{tidx}")
                        nc.scalar.activation(ex_t[:], sc_pair[:], AF.Exp)
                        for back in (1, 0):
                            t2 = tidx - back
                            pending.append(
                                (
                                    slot_of[t2],
                                    first_of[t2],
                                    last_of[t2],
                                    ex_t,
                                    t2 % 2,
                                    xrg_ring[(t2 // 4) % NRG],
                                    t2 % 4,
                                )
                            )
                        while len(pending) > 4:
                            emit_pooled(*pending.popleft())
                    tidx += 1
            while pending:
                emit_pooled(*pending.popleft())
            pump_extracts(0)

        # w1 load — host-pre-tiled contiguous; 4 chunks to avoid hogging DMA
        for c in range(4):
            w = 16 * H
            nc.scalar.dma_start(
                w1_sb[:, c * w : (c + 1) * w], w1_d[:, c * w : (c + 1) * w]
            )

        # ---- MLP tail ----------------------------------------------------
        with (
            tc.tile_pool(name="mlpp", bufs=2, space="PSUM") as mp,
            tc.tile_pool(name="mlps", bufs=2) as ms,
        ):
            h1pp = mp.tile([128, H], f32, tag="h1pp")
            for j in range(64):
                q = j % 4
                nc.tensor.matmul(
                    h1pp[q * 32 : q * 32 + NG, :],
                    P2[:, j * NG : (j + 1) * NG],
                    w1_sb[:, j * H : (j + 1) * H],
                    start=(j < 4),
                    stop=(j >= 60),
                    tile_position=(0, q * 32),
                    skip_group_check=True,
                )
            h1ps = ms.tile([128, H], bf16, tag="h1ps")
            nc.gpsimd.memset(h1ps[:], 0.0)
            for q in range(4):
                if q % 2 == 0:
                    nc.vector.tensor_copy(
                        h1ps[q * 32 : q * 32 + NG, :], h1pp[q * 32 : q * 32 + NG, :]
                    )
                else:
                    nc.scalar.activation(
                        h1ps[q * 32 : q * 32 + NG, :],
                        h1pp[q * 32 : q * 32 + NG, :],
                        AF.Copy,
                    )
            h1p = mp.tile([NG, H], f32, tag="h1p")
            nc.tensor.matmul(h1p[:], qsel_sb[:], h1ps[:], start=True, stop=True)
            h1s = ms.tile([NG, H], f32, tag="h1s")
            nc.vector.tensor_add(h1s[:], h1p[:], b1_sb[:])
            h1g = ms.tile([NG, H], f32, tag="h1g")
            nc.scalar.activation(h1g[:], h1s[:], AF.Sigmoid)
            h1b = ms.tile([NG, H], bf16, tag="h1b")
            nc.vector.tensor_mul(h1b[:], h1s[:], h1g[:])
            h1t = []
            for m in range(2):
                h1tp = mp.tile([128, NG], bf16, tag="h1tp", name=f"h1tp{m}")
                nc.tensor.transpose(
                    h1tp[:], h1b[:, m * 128 : (m + 1) * 128], ident_sb[0:NG, 0:NG]
                )
                ht = ms.tile([128, NG], bf16, tag=f"h1t{m}")
                nc.vector.tensor_copy(ht[:], h1tp[:])
                h1t.append(ht)
            outp = mp.tile([NG, H], f32, tag="outp")
            for k in range(2):
                nc.tensor.matmul(
                    outp[:],
                    h1t[k][:, 0:NG],
                    w2_sb[:, k * H : (k + 1) * H],
                    start=(k == 0),
                    stop=(k == 1),
                )
            osb = ms.tile([NG, H], f32, tag="osb")
            nc.vector.tensor_add(osb[:], outp[:], b2_sb[:])
            nc.sync.dma_start(out_d[:], osb[:])

    return nc


def get_program(slot_tiles: tuple[int, ...]) -> "bass.Bass":
    if slot_tiles not in _PROGRAM_CACHE:
        nc = build_program(slot_tiles)
        # HW-path only (CoreSim snapshots the program before this pass)
        _fix_excess_waits(nc)
        _PROGRAM_CACHE[slot_tiles] = nc
    return _PROGRAM_CACHE[slot_tiles]


# ---------------------------------------------------------------------------
# Host-side sharding / padding


def plan_shards(batch: np.ndarray):
    """Returns (assign [NCORES][NG] graph ids, slot_tiles tuple, sizes)."""
    sizes = np.bincount(batch, minlength=B).astype(np.int64)
    order = np.argsort(-sizes, kind="stable")
    assign = [[] for _ in range(NCORES)]
    for r in range(NG):
        row = order[r * NCORES : (r + 1) * NCORES]
        if r % 2 == 1:
            row = row[::-1]
        for c in range(NCORES):
            assign[c].append(int(row[c]))
    for c in range(NCORES):
        assign[c].sort(key=lambda g: -sizes[g])
    slot_tiles = []
    for j in range(NG):
        mx = max(sizes[assign[c][j]] for c in range(NCORES))
        slot_tiles.append(int(max(1, -(-mx // TILE))))
    # round total tiles up to a group multiple (pad goes to the last slot)
    rem = (-sum(slot_tiles)) % 4
    slot_tiles[-1] += rem
    return assign, tuple(slot_tiles), sizes


def make_in_maps(edge_features, batch, seed_vectors, w_q, w_k, w_v, w1, b1, w2, b2):
    edge_features = np.asarray(edge_features, dtype=np.float32)
    batch = np.asarray(batch)
    assign, slot_tiles, sizes = plan_shards(batch)
    TT = sum(slot_tiles)
    EC = TT * TILE
    NGRP = TT // 4
    XRW = H + 1

    starts = np.searchsorted(batch, np.arange(B))
    xb = edge_features.astype(BF16)

    # Ws[hin, h*S+s] = sum_d w_k[hin, h*HD+d] * q[s, h, d] / sqrt(HD)
    q = (np.asarray(seed_vectors, np.float32) @ np.asarray(w_q, np.float32)).reshape(
        S, NH, HD
    )
    wk3 = np.asarray(w_k, np.float32).reshape(H, NH, HD)
    Ws = (np.einsum("ihd,shd->ihs", wk3, q) * SCALE).reshape(H, NH * S)
    # ws_sb[p, k*256 + c] = Ws[k*128 + p, c]
    ws_host = np.ascontiguousarray(
        Ws.astype(BF16).reshape(2, 128, 256).transpose(1, 0, 2).reshape(128, 512)
    )
    wv = np.asarray(w_v, np.float32).astype(BF16)
    # wvq_sb[p, (k*2+m)*128 + c] = wv[k*128 + p, m*128 + c]
    wvq = np.zeros((128, 4 * 128), dtype=BF16)
    for k in range(2):
        for m in range(2):
            wvq[:, (k * 2 + m) * 128 : (k * 2 + m + 1) * 128] = wv[
                k * 128 : (k + 1) * 128, m * 128 : (m + 1) * 128
            ]
    w1a = np.asarray(w1, np.float32).astype(BF16)
    w1_host = np.ascontiguousarray(
        w1a.reshape(64, 128, H).transpose(1, 0, 2).reshape(128, 64 * H)
    )
    w2a = np.asarray(w2, np.float32).astype(BF16)
    w2_host = np.ascontiguousarray(
        w2a.reshape(2, 128, H).transpose(1, 0, 2).reshape(128, 2 * H)
    )

    shared = {
        "ws": ws_host,
        "wvq": np.ascontiguousarray(wvq),
        "w1": w1_host,
        "w2": w2_host,
        "b1": np.ascontiguousarray(
            np.broadcast_to(np.asarray(b1, dtype=np.float32), (NG, H))
        ),
        "b2": np.ascontiguousarray(
            np.broadcast_to(np.asarray(b2, dtype=np.float32), (NG, H))
        ),
        "ident": np.eye(128, dtype=BF16),
        "qsel": np.ascontiguousarray(
            (np.arange(128)[:, None] % 32 == np.arange(NG)[None, :]).astype(BF16)
        ),
    }

    in_maps = []
    for c in range(NCORES):
        # per-core edge matrix [EC, 256] (rows = padded edge stream)
        xrows = np.zeros((EC, H), dtype=BF16)
        npad = np.zeros(NG, dtype=np.float32)
        off = 0
        for j, g in enumerate(assign[c]):
            n = int(sizes[g])
            xrows[off : off + n] = xb[starts[g] : starts[g] + n]
            npad[j] = slot_tiles[j] * TILE - n
            off += slot_tiles[j] * TILE
        # xt: [128, NGRP*1024]; xt[p, grp*1024 + k*512 + c] = x[grp*512+c, k*128+p]
        xt4 = xrows.reshape(NGRP, 512, 2, 128)  # [grp, c, k, p]
        xt_host = np.ascontiguousarray(
            xt4.transpose(3, 0, 2, 1).reshape(128, NGRP * 1024)
        )
        # xr: [128, NGRP*4*257]; xr[p, ((grp*4+t)*257)+c] = x[grp*512+t*128+p, c]
        xr5 = xrows.reshape(NGRP, 4, 128, H)  # [grp, t, p, c]
        xr_host = np.ones((128, NGRP, 4, XRW), dtype=BF16)
        xr_host[:, :, :, 0:H] = xr5.transpose(2, 0, 1, 3)
        m = dict(shared)
        m["xt"] = xt_host
        m["xr"] = np.ascontiguousarray(xr_host.reshape(128, NGRP * 4 * XRW))
        m["npad"] = np.ascontiguousarray(np.broadcast_to(npad, (128, NG)))
        in_maps.append(m)
    return in_maps, assign, slot_tiles


def kernel(
    edge_features,
    edge_coords,
    batch,
    seed_vectors,
    w_q,
    w_k,
    w_v,
    w1,
    b1,
    w2,
    b2,
):
    in_maps, assign, slot_tiles = make_in_maps(
        edge_features, batch, seed_vectors, w_q, w_k, w_v, w1, b1, w2, b2
    )
    nc = get_program(slot_tiles)

    res = run_bass_kernel_spmd(nc, in_maps, core_ids=list(range(NCORES)))
    global LAST_RESULTS
    LAST_RESULTS = res

    out = np.zeros((B, H), dtype=np.float32)
    for c in range(NCORES):
        o = res.results[c]["out"]  # [NG, H]
        for j, g in enumerate(assign[c]):
            out[g, :] = o[j, :]
    return out


# revision 29
# speedup vs baseline: 1.5304x; 1.0873x over previous
"""AttentionPooling (ragged graph cross-attention pooling) on 8 TRN2 NeuronCores.

v2 strategy (SPMD, no collectives) — "x-pooling" restructure:
  * Host assigns 8 whole graphs per core (serpentine by size), sorted into 8
    slots; per-slot tile counts are shared across cores (shared instruction
    stream); edges zero-padded to the slot size.
  * Linearity trick: pooled_v = (sum_e w[e,s,h] * x_e) @ w_v — pool the RAW
    edge features with the attention weights and apply w_v once per slot on
    the tiny pooled matrix.  This removes the per-edge V projection (PE) and
    the per-tile PSUM->SBUF V copy (DVE) entirely.
  * Per 128-edge tile the device does only:
      scores = x^T-tile @ Ws           (PE, 2 matmuls N=256, psum [e,256])
      ex     = exp(scores)             (ACT, one [128,512] EXP per 2 tiles)
      xpool += ex^T @ [x | 1]          (PE, 2 matmuls N=257, psum-accum/slot)
    with Ws = w_k @ q^T / sqrt(hd) host-folded (no separate K projection) and
    a 1s column baked into the x stream producing the softmax denominator
    (host-computed npad corrects for padding; pad edges have x=0, exp(0)=1).
  * Per slot (split into stages so the FIFO PE queue never waits on DVE):
    normalize by 1/(denom-npad) + cast bf16 (DVE), 4 PE transposes, apply the
    four w_v quadrants (4 matmuls N=128) -> pov^T [hd, sh], scatter the
    block-diagonal seed/head blocks into the MLP operand P2.
  * MLP: h1pre j-loop over 64 blocks, 4-way tile_position-packed into TWO
    psum banks (strip evacuation runs DVE || ACT), bias b1 rides the qsel
    combine matmul as an extra contraction row, silu computed from the
    already-resident exp table (x*sigmoid(x) = x/(1+exp(-x))) to avoid an ACT
    table swap, out = h1 @ w2 + b2 emitted row-contiguous [NG, H].
  * All bulk DMA is host-pre-tiled so every transfer is >=2KB contiguous per
    partition with ONE trigger per group per stream; w1 dribbles in 32 small
    chunks mid-loop; the xr (pooling) stream lags xt by one group since
    scores consume first.
  * Dummy matmul chains at kernel start and through the tail keep the PE
    p-state at full clock (2.4GHz) across unavoidable dependency stalls.
"""

import os
import sys
from collections import deque
from contextlib import ExitStack

import numpy as np

for _p in ("/opt/trn_rl_repo",):
    if _p not in sys.path:
        sys.path.append(_p)

import ml_dtypes  # noqa: E402

import concourse.bass as bass  # noqa: E402
import concourse.tile as tile  # noqa: E402
from concourse import mybir  # noqa: E402
from concourse.bass_utils import run_bass_kernel_spmd  # noqa: E402
from concourse.vector_clock import ScopedClock  # noqa: E402

BF16 = ml_dtypes.bfloat16

E, B, H, S, NH, HD = 131072, 64, 256, 32, 8, 32
NCORES = 8
NG = B // NCORES        # graphs (slots) per core
TILE = 128              # edge tile
GROUP = 512             # edges per DMA group (4 tiles)
SCALE = 1.0 / float(np.sqrt(HD))
WARM_MM = 8             # PE p-state warmup matmuls

AF = mybir.ActivationFunctionType

# ---------------------------------------------------------------------------
# Walrus workaround: this toolchain's InstDrain accepts only ONE sync wait;
# Tile's kernel-tail drain carries one wait per outstanding semaphore.
# Split it into a chain of single-wait drains.
_MAXW = 1


def _split_drain_and_barrier(self, tick_clock, wait_clock):
    nc = self.nc
    drain_inst = nc.sync.drain()
    wait_clock.add_sem_waits(
        drain_inst.ins, ScopedClock({None: tick_clock.global_clock})
    )
    waits = list(drain_inst.ins.sync_info.on_wait)
    if len(waits) > _MAXW:
        drain_inst.ins.sync_info = mybir.SyncInfo(on_wait=waits[:_MAXW], on_update=[])
        for i in range(_MAXW, len(waits), _MAXW):
            d2 = nc.sync.drain()
            d2.ins.sync_info = mybir.SyncInfo(
                on_wait=waits[i : i + _MAXW], on_update=[]
            )
    nc.all_engine_barrier()
    popped = nc._tile_sem_poison_stack.pop()
    assert popped is self._sem_poison
    nc.clear_and_free_semaphores(list(self.sems.allocated().values()))
    nc.all_engine_barrier()


tile.TileContext._drain_and_barrier = _split_drain_and_barrier

# Engine instructions are capped at 2 sync waits by this walrus (Drain/NoOp
# at 1).  Tile's sem-assignment occasionally emits more.  Hoist the excess
# onto single-wait NoOps inserted just before, on the same engine.
_WAIT_CAP = {"InstDrain": 1}
_WAIT_CAP_DEFAULT = 1


def _fix_excess_waits(nc):
    n_fixed = 0
    for fn in nc.m.functions:
        for bb in fn.blocks:
            insts = bb.instructions
            out = []
            changed = False
            for inst in insts:
                si = inst.sync_info
                waits = list(si.on_wait) if si is not None else []
                cap = _WAIT_CAP.get(type(inst).__name__, _WAIT_CAP_DEFAULT)
                if len(waits) > cap:
                    changed = True
                    n_fixed += 1
                    excess = waits[: len(waits) - cap]
                    for i, w in enumerate(excess):
                        nop = mybir.InstNoOp(
                            name=f"{inst.name}-hw{i}", ins=[], outs=[]
                        )
                        nop.engine = inst.engine
                        nop.sync_info = mybir.SyncInfo(on_wait=[w], on_update=[])
                        out.append(nop)
                    inst.sync_info = mybir.SyncInfo(
                        on_wait=waits[len(excess) :], on_update=list(si.on_update)
                    )
                out.append(inst)
            if changed:
                bb.instructions = out
    return n_fixed


# ---------------------------------------------------------------------------

_PROGRAM_CACHE: dict[tuple, "bass.Bass"] = {}
LAST_RESULTS = None  # BassKernelResults of the most recent run (for testing)


def _install_ntff_hook_shim():
    """The image's antenv lacks axon_hooks; recreate it so trace=True works."""
    try:
        import types

        import antenv

        if "antenv.axon_hooks" not in sys.modules:
            mod = types.ModuleType("antenv.axon_hooks")
            mod._hook = None

            def set_axon_ntff_profile_hook(h):
                mod._hook = h

            def get_axon_ntff_profile_hook():
                return mod._hook

            mod.set_axon_ntff_profile_hook = set_axon_ntff_profile_hook
            mod.get_axon_ntff_profile_hook = get_axon_ntff_profile_hook
            sys.modules["antenv.axon_hooks"] = mod
            antenv.axon_hooks = mod
        import antenv.axon_hooks as ah

        if ah.get_axon_ntff_profile_hook() is None:
            from trn_agent_boot.trn_boot import _ntff_profile_via_ctypes

            ah.set_axon_ntff_profile_hook(
                _ntff_profile_via_ctypes("/opt/axon/libaxon_pjrt.so")
            )
    except Exception:
        pass


_install_ntff_hook_shim()


def build_program(slot_tiles: tuple[int, ...]) -> "bass.Bass":
    """Build the SPMD Bass program for per-core slot tile counts."""
    TT = sum(slot_tiles)
    assert TT % 4 == 0
    NGRP = TT // 4

    # per-tile slot id / first / last flags
    slot_of, first_of, last_of = [], [], []
    for j, tj in enumerate(slot_tiles):
        for t in range(tj):
            slot_of.append(j)
            first_of.append(t == 0)
            last_of.append(t == tj - 1)

    f32, bf16 = mybir.dt.float32, mybir.dt.bfloat16
    nc = bass.Bass("TRN2", target_bir_lowering=False, debug=False, num_devices=NCORES)

    # host-pre-tiled inputs (all >=2KB contiguous per partition per group)
    xt_d = nc.dram_tensor("xt", [128, NGRP * 1024], bf16, kind="ExternalInput").ap()
    xr_d = nc.dram_tensor("xr", [128, NGRP * 4 * (H + 1)], bf16, kind="ExternalInput").ap()
    ws_d = nc.dram_tensor("ws", [128, 2 * H], bf16, kind="ExternalInput").ap()
    wvq_d = nc.dram_tensor("wvq", [128, 4 * 128], bf16, kind="ExternalInput").ap()
    w1_d = nc.dram_tensor("w1", [128, 64 * H], bf16, kind="ExternalInput").ap()
    w2_d = nc.dram_tensor("w2", [128, 2 * H], bf16, kind="ExternalInput").ap()
    b1_d = nc.dram_tensor("b1", [32, H], f32, kind="ExternalInput").ap()
    b2_d = nc.dram_tensor("b2", [NG, H], f32, kind="ExternalInput").ap()
    npad_d = nc.dram_tensor("npad", [128, NG], f32, kind="ExternalInput").ap()
    ident_d = nc.dram_tensor("ident", [128, 128], bf16, kind="ExternalInput").ap()
    qsel_d = nc.dram_tensor("qsel", [128, NG], bf16, kind="ExternalInput").ap()
    out_d = nc.dram_tensor("out", [NG, H], f32, kind="ExternalOutput").ap()

    XRW = H + 1  # 257: x tile width incl. baked-in 1s column

    with tile.TileContext(nc) as tc, ExitStack() as ctx:
        const = ctx.enter_context(tc.tile_pool(name="const", bufs=1))
        ws_sb = const.tile([128, 2 * H], bf16)
        wvq_sb = const.tile([128, 4 * 128], bf16)
        w1_sb = const.tile([128, 64 * H], bf16)
        w2_sb = const.tile([128, 2 * H], bf16)
        ident_sb = const.tile([128, 128], bf16)
        qsel_sb = const.tile([128, NG], bf16)
        b1_sb = const.tile([32, H], f32)
        b2_sb = const.tile([NG, H], f32)
        npad_sb = const.tile([128, NG], f32)
        P2 = const.tile([128, 32 * 2 * NG], bf16)

        # PE p-state warmup: a chain of dummy matmuls keeps the PE busy (and
        # ramping to full clock) while the first input DMAs are in flight.
        wz = const.tile([128, 512], bf16)
        nc.gpsimd.memset(wz[:], 0.0)

        # first-needed consts on the scalar DGE ring
        nc.scalar.dma_start(ws_sb[:], ws_d[:])
        nc.scalar.dma_start(npad_sb[:], npad_d[:])

        # ACT table warm (exp + sigmoid) while DMAs fly
        warm = const.tile([1, 2], f32)
        nc.gpsimd.memset(warm[:, 0:1], 0.0)
        nc.scalar.activation(warm[:, 1:2], warm[:, 0:1], AF.Exp)

        nc.scalar.dma_start(wvq_sb[:], wvq_d[:])
        nc.scalar.dma_start(ident_sb[:], ident_d[:])

        with tc.tile_pool(name="warmp", bufs=1, space="PSUM") as wp_pool:
            wp = wp_pool.tile([128, 512], f32)
            for i in range(WARM_MM):
                nc.tensor.matmul(wp[:], wz[:, 0:128], wz[:], start=True, stop=True)

        # input rings (manual, so buffers persist and deps are per-buffer)
        NRG = 5  # groups in flight
        xtg_ring = [const.tile([128, 2, 512], bf16, name=f"xtg{i}") for i in range(NRG)]
        xrg_ring = [
            const.tile([128, 4, XRW], bf16, name=f"xrg{i}") for i in range(NRG)
        ]

        ex_pool = ctx.enter_context(tc.tile_pool(name="exp", bufs=5))
        ext_pool = ctx.enter_context(tc.tile_pool(name="ext", bufs=2))

        xp_tiles: list = [None, None]

        with (
            tc.tile_pool(name="scp", bufs=2, space="PSUM") as sc_pool,
            tc.tile_pool(name="xpp", bufs=2, space="PSUM") as xp_pool,
            tc.tile_pool(name="tpp", bufs=1, space="PSUM") as tp_pool,
            tc.tile_pool(name="pvp", bufs=1, space="PSUM") as pv_pool,
        ):
            P2v = P2[:].rearrange("p (s x) -> p s x", x=2 * NG)

            def extract_stage1(g, xp):
                """Per-slot DVE work right after the slot's last pooling MM:
                denominator, reciprocal, normalize+cast."""
                recs, pns = [], []
                for m in range(2):
                    den = ext_pool.tile([128, 1], f32, tag="den", name=f"den{g}_{m}")
                    nc.vector.tensor_scalar_sub(
                        den[:], xp[m][:, H : H + 1], npad_sb[:, g : g + 1]
                    )
                    rec = ext_pool.tile([128, 1], f32, tag="rec", name=f"rec{g}_{m}")
                    nc.vector.reciprocal(rec[:], den[:])
                    recs.append(rec)
                for m in range(2):
                    pn = ext_pool.tile([128, 256], bf16, tag=f"pn{m}", name=f"pn{g}_{m}")
                    nc.vector.tensor_scalar_mul(pn[:], xp[m][:, 0:256], recs[m][:])
                    pns.append(pn)
                return pns

            def extract_stage2(g, pns):
                """Deferred PE work (so the FIFO PE queue never waits on the
                DVE normalize): transpose, apply w_v quadrants, scatter P2."""
                tps = tp_pool.tile([128, 512], bf16, tag="tps", name=f"tps{g}")
                for m in range(2):
                    for k in range(2):
                        q = m * 2 + k
                        nc.tensor.transpose(
                            tps[:, q * 128 : (q + 1) * 128],
                            pns[m][:, k * 128 : (k + 1) * 128],
                            ident_sb[:],
                        )
                xpT = []
                for m in range(2):
                    row = []
                    for k in range(2):
                        q = m * 2 + k
                        t_sb = ext_pool.tile(
                            [128, 128], bf16, tag=f"xpT{q}", name=f"xpT{g}_{q}"
                        )
                        nc.vector.tensor_copy(t_sb[:], tps[:, q * 128 : (q + 1) * 128])
                        row.append(t_sb)
                    xpT.append(row)
                return g, xpT

            def extract_stage3(g, xpT):
                pov = pv_pool.tile([128, 256], f32, tag="pov", name=f"pov{g}")
                for m in range(2):
                    for k in range(2):
                        nc.tensor.matmul(
                            pov[:, m * 128 : (m + 1) * 128],
                            wvq_sb[:, (k * 2 + m) * 128 : (k * 2 + m + 1) * 128],
                            xpT[m][k][:],
                            start=(k == 0),
                            stop=(k == 1),
                        )
                pv_sb = ext_pool.tile([128, 256], bf16, tag="pv", name=f"pv{g}")
                nc.vector.tensor_copy(pv_sb[:], pov[:])
                for m in range(2):
                    for hh in range(4):
                        if g == NG - 1:
                            # final slot gates the MLP: split across engines
                            copy_eng = nc.vector if (m * 4 + hh) % 2 == 0 else nc.gpsimd
                        else:
                            copy_eng = nc.gpsimd
                        rr = slice(hh * 32, (hh + 1) * 32)
                        src = pv_sb[
                            rr, m * 128 + hh * 32 : m * 128 + (hh + 1) * 32
                        ].rearrange("p (a o) -> p a o", o=1)
                        copy_eng.tensor_copy(P2v[rr, :, m * NG + g : m * NG + g + 1], src)

            npooled = 0
            ext_queue = deque()  # (emitted-at-count, stage, payload)

            def pump_extracts(limit):
                while ext_queue and npooled - ext_queue[0][0] >= limit:
                    at, stage, payload = ext_queue.popleft()
                    if stage == 1:
                        g, pns = payload
                        ext_queue.append((npooled, 2, extract_stage2(g, pns)))
                    else:
                        extract_stage3(*payload)

            def emit_pooled(sl, fi, la, ex_t, half, xr_t, sub):
                nonlocal npooled
                if fi:
                    xp_tiles[0] = xp_pool.tile(
                        [128, H + 1], f32, tag="xp0", name=f"xp0_s{sl}"
                    )
                    xp_tiles[1] = xp_pool.tile(
                        [128, H + 1], f32, tag="xp1", name=f"xp1_s{sl}"
                    )
                for m in range(2):
                    nc.tensor.matmul(
                        xp_tiles[m][:],
                        ex_t[:, half * 256 + m * 128 : half * 256 + (m + 1) * 128],
                        xr_t[:, sub, :],
                        start=fi,
                        stop=la,
                    )
                npooled += 1
                if la:
                    pns = extract_stage1(sl, xp_tiles)
                    ext_queue.append((npooled, 1, (sl, pns)))
                pump_extracts(3)

            pending = deque()
            sc_pair = None
            tidx = 0
            w1_nchunk = 32
            w1_at = {}
            for c in range(w1_nchunk):
                g_tgt = 5 + (c * max(1, NGRP - 10)) // w1_nchunk
                w1_at.setdefault(min(g_tgt, NGRP - 1), []).append(c)
            for grp in range(NGRP):
                xtg = xtg_ring[grp % NRG]
                xrg = xrg_ring[grp % NRG]
                nc.sync.dma_start(
                    xtg[:],
                    xt_d[:, grp * 1024 : (grp + 1) * 1024].rearrange(
                        "p (k c) -> p k c", k=2
                    ),
                )
                for gx in ([grp - 1] if grp > 0 else []) + (
                    [grp] if grp == NGRP - 1 else []
                ):
                    nc.sync.dma_start(
                        xrg_ring[gx % NRG][:],
                        xr_d[:, gx * 4 * XRW : (gx + 1) * 4 * XRW].rearrange(
                            "p (t c) -> p t c", t=4
                        ),
                    )
                # w1 dribbles in small chunks mid-loop: after the DMA-bound
                # start, well before the MLP needs it
                for c in w1_at.get(grp, ()):
                    w = 2 * H
                    nc.scalar.dma_start(
                        w1_sb[:, c * w : (c + 1) * w], w1_d[:, c * w : (c + 1) * w]
                    )
                if grp == 4:
                    nc.scalar.dma_start(qsel_sb[:], qsel_d[:])
                    nc.scalar.dma_start(b1_sb[:], b1_d[:])
                if grp == 5:
                    nc.scalar.dma_start(b2_sb[:], b2_d[:])
                    nc.scalar.dma_start(w2_sb[:], w2_d[:])
                for sub in range(4):
                    half = tidx % 2
                    if half == 0:
                        sc_pair = sc_pool.tile(
                            [128, 512], f32, tag="sc", name=f"sc{tidx}"
                        )
                    for k in range(2):
                        nc.tensor.matmul(
                            sc_pair[:, half * 256 : (half + 1) * 256],
                            xtg[:, k, sub * TILE : (sub + 1) * TILE],
                            ws_sb[:, k * 256 : (k + 1) * 256],
                            start=(k == 0),
                            stop=(k == 1),
                        )
                    if half == 1:
                        ex_t = ex_pool.tile([128, 512], bf16, tag="ex", name=f"ex{tidx}")
                        nc.scalar.activation(ex_t[:], sc_pair[:], AF.Exp)
                        for back in (1, 0):
                            t2 = tidx - back
                            pending.append(
                                (
                                    slot_of[t2],
                                    first_of[t2],
                                    last_of[t2],
                                    ex_t,
                                    t2 % 2,
                                    xrg_ring[(t2 // 4) % NRG],
                                    t2 % 4,
                                )
                            )
                        while len(pending) > 8:
                            emit_pooled(*pending.popleft())
                    tidx += 1
            while pending:
                emit_pooled(*pending.popleft())

            def pe_filler(n):
                wsc = sc_pool.tile([128, 512], f32, tag="sc", name=f"fill{npooled}_{n}")
                for _ in range(n):
                    nc.tensor.matmul(wsc[:], wz[:, 0:128], wz[:], start=True, stop=True)

            # flush remaining extracts, keeping the PE p-state hot with dummy
            # matmuls while the DVE stages of the final slots complete
            while ext_queue:
                at, stage, payload = ext_queue.popleft()
                if stage == 1:
                    pe_filler(4)
                    g, pns = payload
                    ext_queue.append((npooled, 2, extract_stage2(g, pns)))
                else:
                    pe_filler(3)
                    extract_stage3(*payload)
            pe_filler(7)

        # ---- MLP tail ----------------------------------------------------
        with (
            tc.tile_pool(name="mlpp", bufs=1, space="PSUM") as mp,
            tc.tile_pool(name="mlps", bufs=2) as ms,
        ):
            h1ppA = mp.tile([128, H], f32, tag="h1ppA")
            h1ppB = mp.tile([128, H], f32, tag="h1ppB")
            wmp = mp.tile([128, 512], f32, tag="fill")

            def mlp_filler(n):
                for _ in range(n):
                    nc.tensor.matmul(wmp[:], wz[:, 0:128], wz[:], start=True, stop=True)

            for j in range(64):
                q = j % 4
                dst = h1ppA if q < 2 else h1ppB
                r0 = q * 32
                nc.tensor.matmul(
                    dst[r0 : r0 + NG, :],
                    P2[:, j * NG : (j + 1) * NG],
                    w1_sb[:, j * H : (j + 1) * H],
                    start=(j < 4),
                    stop=(j >= 60),
                    tile_position=(0, q * 32),
                    skip_group_check=True,
                )
            mlp_filler(4)
            h1ps = ms.tile([128, H], bf16, tag="h1ps")
            nc.gpsimd.memset(h1ps[:], 0.0)
            # 32-aligned block carrying b1 at row NG; strip q0 then overwrites
            # rows 0:NG with real data
            nc.vector.tensor_copy(h1ps[0:32, :], b1_sb[:])
            # A strips on DVE, B strips on ACT — different psum banks, parallel
            for q in range(2):
                nc.vector.tensor_copy(
                    h1ps[q * 32 : q * 32 + NG, :], h1ppA[q * 32 : q * 32 + NG, :]
                )
                r0 = (q + 2) * 32
                nc.scalar.activation(
                    h1ps[r0 : r0 + NG, :], h1ppB[r0 : r0 + NG, :], AF.Copy
                )
            h1p = mp.tile([NG, H], f32, tag="h1p")
            nc.tensor.matmul(h1p[:], qsel_sb[:], h1ps[:], start=True, stop=True)
            mlp_filler(6)
            # silu via the already-resident exp table: x / (1 + exp(-x));
            # the +b1 bias rode along in the qsel contraction (h1ps row NG)
            en = ms.tile([NG, H], f32, tag="en")
            nc.scalar.activation(en[:], h1p[:], AF.Exp, scale=-1.0)
            ed = ms.tile([NG, H], f32, tag="ed")
            nc.vector.tensor_scalar_add(ed[:], en[:], 1.0)
            er = ms.tile([NG, H], f32, tag="er")
            nc.vector.reciprocal(er[:], ed[:])
            h1b = ms.tile([NG, H], bf16, tag="h1b")
            nc.vector.tensor_mul(h1b[:], h1p[:], er[:])
            h1t = []
            for m in range(2):
                h1tp = mp.tile([128, NG], bf16, tag="h1tp", name=f"h1tp{m}")
                nc.tensor.transpose(
                    h1tp[:], h1b[:, m * 128 : (m + 1) * 128], ident_sb[0:NG, 0:NG]
                )
                ht = ms.tile([128, NG], bf16, tag=f"h1t{m}")
                nc.vector.tensor_copy(ht[:], h1tp[:])
                h1t.append(ht)
            mlp_filler(1)
            outp = mp.tile([NG, H], f32, tag="outp")
            for k in range(2):
                nc.tensor.matmul(
                    outp[:],
                    h1t[k][:, 0:NG],
                    w2_sb[:, k * H : (k + 1) * H],
                    start=(k == 0),
                    stop=(k == 1),
                )
            osb = ms.tile([NG, H], f32, tag="osb")
            nc.vector.tensor_add(osb[:], outp[:], b2_sb[:])
            nc.sync.dma_start(out_d[:], osb[:])

    return nc


def get_program(slot_tiles: tuple[int, ...]) -> "bass.Bass":
    if slot_tiles not in _PROGRAM_CACHE:
        nc = build_program(slot_tiles)
        # HW-path only (CoreSim snapshots the program before this pass)
        _fix_excess_waits(nc)
        _PROGRAM_CACHE[slot_tiles] = nc
    return _PROGRAM_CACHE[slot_tiles]


# ---------------------------------------------------------------------------
# Host-side sharding / padding


def plan_shards(batch: np.ndarray):
    """Returns (assign [NCORES][NG] graph ids, slot_tiles tuple, sizes)."""
    sizes = np.bincount(batch, minlength=B).astype(np.int64)
    order = np.argsort(-sizes, kind="stable")
    assign = [[] for _ in range(NCORES)]
    for r in range(NG):
        row = order[r * NCORES : (r + 1) * NCORES]
        if r % 2 == 1:
            row = row[::-1]
        for c in range(NCORES):
            assign[c].append(int(row[c]))
    for c in range(NCORES):
        assign[c].sort(key=lambda g: -sizes[g])
    slot_tiles = []
    for j in range(NG):
        mx = max(sizes[assign[c][j]] for c in range(NCORES))
        slot_tiles.append(int(max(1, -(-mx // TILE))))
    # round total tiles up to a group multiple (pad goes to the last slot)
    rem = (-sum(slot_tiles)) % 4
    slot_tiles[-1] += rem
    return assign, tuple(slot_tiles), sizes


def _make_b1_block(b1):
    blk = np.zeros((32, H), dtype=np.float32)
    blk[NG, :] = np.asarray(b1, dtype=np.float32)
    return blk


def _make_qsel():
    q = (np.arange(128)[:, None] % 32 == np.arange(NG)[None, :]).astype(np.float32)
    q[np.arange(128) % 32 >= NG] = 0.0
    q[NG, :] = 1.0  # b1 bias row rides along in the contraction
    return np.ascontiguousarray(q.astype(BF16))


def make_in_maps(edge_features, batch, seed_vectors, w_q, w_k, w_v, w1, b1, w2, b2):
    edge_features = np.asarray(edge_features, dtype=np.float32)
    batch = np.asarray(batch)
    assign, slot_tiles, sizes = plan_shards(batch)
    TT = sum(slot_tiles)
    EC = TT * TILE
    NGRP = TT // 4
    XRW = H + 1

    starts = np.searchsorted(batch, np.arange(B))
    xb = edge_features.astype(BF16)

    # Ws[hin, h*S+s] = sum_d w_k[hin, h*HD+d] * q[s, h, d] / sqrt(HD)
    q = (np.asarray(seed_vectors, np.float32) @ np.asarray(w_q, np.float32)).reshape(
        S, NH, HD
    )
    wk3 = np.asarray(w_k, np.float32).reshape(H, NH, HD)
    Ws = (np.einsum("ihd,shd->ihs", wk3, q) * SCALE).reshape(H, NH * S)
    # ws_sb[p, k*256 + c] = Ws[k*128 + p, c]
    ws_host = np.ascontiguousarray(
        Ws.astype(BF16).reshape(2, 128, 256).transpose(1, 0, 2).reshape(128, 512)
    )
    wv = np.asarray(w_v, np.float32).astype(BF16)
    # wvq_sb[p, (k*2+m)*128 + c] = wv[k*128 + p, m*128 + c]
    wvq = np.zeros((128, 4 * 128), dtype=BF16)
    for k in range(2):
        for m in range(2):
            wvq[:, (k * 2 + m) * 128 : (k * 2 + m + 1) * 128] = wv[
                k * 128 : (k + 1) * 128, m * 128 : (m + 1) * 128
            ]
    w1a = np.asarray(w1, np.float32).astype(BF16)
    w1_host = np.ascontiguousarray(
        w1a.reshape(64, 128, H).transpose(1, 0, 2).reshape(128, 64 * H)
    )
    w2a = np.asarray(w2, np.float32).astype(BF16)
    w2_host = np.ascontiguousarray(
        w2a.reshape(2, 128, H).transpose(1, 0, 2).reshape(128, 2 * H)
    )

    shared = {
        "ws": ws_host,
        "wvq": np.ascontiguousarray(wvq),
        "w1": w1_host,
        "w2": w2_host,
        "b1": _make_b1_block(b1),
        "b2": np.ascontiguousarray(
            np.broadcast_to(np.asarray(b2, dtype=np.float32), (NG, H))
        ),
        "ident": np.eye(128, dtype=BF16),
        "qsel": _make_qsel(),
    }

    in_maps = []
    for c in range(NCORES):
        # per-core edge matrix [EC, 256] (rows = padded edge stream)
        xrows = np.zeros((EC, H), dtype=BF16)
        npad = np.zeros(NG, dtype=np.float32)
        off = 0
        for j, g in enumerate(assign[c]):
            n = int(sizes[g])
            xrows[off : off + n] = xb[starts[g] : starts[g] + n]
            npad[j] = slot_tiles[j] * TILE - n
            off += slot_tiles[j] * TILE
        # xt: [128, NGRP*1024]; xt[p, grp*1024 + k*512 + c] = x[grp*512+c, k*128+p]
        xt4 = xrows.reshape(NGRP, 512, 2, 128)  # [grp, c, k, p]
        xt_host = np.ascontiguousarray(
            xt4.transpose(3, 0, 2, 1).reshape(128, NGRP * 1024)
        )
        # xr: [128, NGRP*4*257]; xr[p, ((grp*4+t)*257)+c] = x[grp*512+t*128+p, c]
        xr5 = xrows.reshape(NGRP, 4, 128, H)  # [grp, t, p, c]
        xr_host = np.ones((128, NGRP, 4, XRW), dtype=BF16)
        xr_host[:, :, :, 0:H] = xr5.transpose(2, 0, 1, 3)
        m = dict(shared)
        m["xt"] = xt_host
        m["xr"] = np.ascontiguousarray(xr_host.reshape(128, NGRP * 4 * XRW))
        m["npad"] = np.ascontiguousarray(np.broadcast_to(npad, (128, NG)))
        in_maps.append(m)
    return in_maps, assign, slot_tiles


def kernel(
    edge_features,
    edge_coords,
    batch,
    seed_vectors,
    w_q,
    w_k,
    w_v,
    w1,
    b1,
    w2,
    b2,
):
    in_maps, assign, slot_tiles = make_in_maps(
        edge_features, batch, seed_vectors, w_q, w_k, w_v, w1, b1, w2, b2
    )
    nc = get_program(slot_tiles)

    res = run_bass_kernel_spmd(nc, in_maps, core_ids=list(range(NCORES)))
    global LAST_RESULTS
    LAST_RESULTS = res

    out = np.zeros((B, H), dtype=np.float32)
    for c in range(NCORES):
        o = res.results[c]["out"]  # [NG, H]
        for j, g in enumerate(assign[c]):
            out[g, :] = o[j, :]
    return out
